# revision 45
# baseline (speedup 1.0000x reference)
"""Causal multi-head attention with RoPE on 8 Trainium2 NeuronCores.

Sharding: core c -> batch b = c//2, head-group g = c%2 (8 of 16 heads).
Each core computes q/k/v projections for its batch+heads, applies RoPE
(evens/odds row-permuted layout so the pair-rotation becomes a
32-partition-group swap done by SBUF-SBUF DMA), runs flash-style causal
attention with transposed scores (softmax sum via an appended ones
column of V -> denominator row in the AV PSUM tile), and a partial
output projection over its head group. Host sums the two per-batch
partials.

v2: bf16 matmul data (FWL weight loads, halved DMA/SBUF), trapezoid
causal structure (masked 128-col sub-blocks of diagonal tiles skipped
entirely; only the triangular sub-block gets a mask multiply), and the
projection / output-projection matmuls of neighboring quarters are
interleaved into the attention instruction stream so the PE never
idles behind the exp-bound softmax chain.
"""
import math
import sys

sys.path.insert(0, "/opt/trn_rl_repo")

import numpy as np
import ml_dtypes

import concourse.tile as tile
import concourse.bass as bass
from concourse import bacc, mybir
from concourse.bass_utils import run_bass_kernel_spmd

NUM_HEADS = 16
B, S, D = 4, 2048, 1024
HPC = 8            # heads per core
DK = 64
HD = HPC * DK      # 512 head dims per core
THETA = 10000.0
N_CORES = 8
KC = D // 128      # 8 contraction chunks for projections
NQ = 4             # s-quarters of 512
ST = S // 128      # 16 s-tiles

f32 = mybir.dt.float32
bf16 = mybir.dt.bfloat16
ActF = mybir.ActivationFunctionType
Alu = mybir.AluOpType

_prog_cache = {}


def _build_program():
    nc = bacc.Bacc("TRN2", target_bir_lowering=False, debug=False,
                   enable_asserts=False, num_devices=N_CORES)
    xT_d = nc.dram_tensor("xT", [D, S], bf16, kind="ExternalInput").ap()
    wqk_d = nc.dram_tensor("wqkT", [D, 2 * HD], bf16, kind="ExternalInput").ap()
    wv_d = nc.dram_tensor("wvT", [D, HD], bf16, kind="ExternalInput").ap()
    wo_d = nc.dram_tensor("woT", [HD, D], bf16, kind="ExternalInput").ap()
    cos_d = nc.dram_tensor("cosT", [128, S], bf16, kind="ExternalInput").ap()
    sin_d = nc.dram_tensor("sinT", [128, S], bf16, kind="ExternalInput").ap()
    tri_d = nc.dram_tensor("tri", [128, 128], bf16, kind="ExternalInput").ap()
    vone_d = nc.dram_tensor("vones", [128, ST * HPC], bf16, kind="ExternalInput").ap()
    out_d = nc.dram_tensor("outT", [D, S], f32, kind="ExternalOutput").ap()

    with tile.TileContext(nc) as tc:
        with tc.tile_pool(name="persist", bufs=1) as pp, \
             tc.tile_pool(name="xq", bufs=2) as xp, \
             tc.tile_pool(name="s2", bufs=3) as s2p, \
             tc.tile_pool(name="t12", bufs=3) as tp2, \
             tc.tile_pool(name="ex", bufs=6) as exp_pool, \
             tc.tile_pool(name="unrm", bufs=3) as unp, \
             tc.tile_pool(name="nrm", bufs=3) as nrmp, \
             tc.tile_pool(name="ob", bufs=3) as obp, \
             tc.tile_pool(name="nrmd", bufs=6, space="DRAM") as nrmd_pool, \
             tc.tile_pool(name="paux", bufs=2, space="PSUM") as paux, \
             tc.tile_pool(name="pss", bufs=2, space="PSUM") as pss, \
             tc.tile_pool(name="pso", bufs=1, space="PSUM") as pso:

            # ---------------- persistent SBUF ---------------------------
            qkrot = pp.tile([128, KC, S], bf16, tag="qkrot")           # 32KB/P
            v_aug = pp.tile([128, ST, HPC, DK + 1], bf16, tag="v_aug")  # 16.6KB
            attn_outT = pp.tile([128, NQ, S], bf16, tag="attn_outT")   # 16KB
            wqk_sb = pp.tile([128, KC, 2 * HD], bf16, tag="wqk")       # 16KB
            wv_sb = pp.tile([128, KC, HD], bf16, tag="wv")             # 8KB
            wo_sb = pp.tile([128, HD // 128, D], bf16, tag="wo")       # 8KB
            cos_sb = pp.tile([128, S], bf16, tag="cos")                # 4KB
            sin_sb = pp.tile([128, S], bf16, tag="sin")                # 4KB
            tri_sb = pp.tile([128, 128], bf16, tag="tri")

            # warmup burst: ~6us of dummy matmuls on zeros so the PE HAM
            # clock gate opens (1.2 -> 2.4 GHz) while the first x/weight
            # DMAs are still in flight
            warm_sb = pp.tile([128, 512], bf16, tag="warm")
            nc.gpsimd.memset(warm_sb[:], 0.0)
            for _ in range(14):
                psw = paux.tile([128, 512], f32, tag="paux", name="psw")
                nc.tensor.matmul(psw[:], warm_sb[:, 0:128], warm_sb[:],
                                 start=True, stop=True)

            xq_tiles = {}

            def load_xq(q):
                xq = xp.tile([128, KC, 512], bf16, tag="xq", name=f"xq{q}")
                sl = slice(q * 512, (q + 1) * 512)
                for k in range(KC):
                    nc.gpsimd.dma_start(out=xq[:, k, :], in_=xT_d[k * 128:(k + 1) * 128, sl])
                xq_tiles[q] = xq

            def v_task(q, m):
                st = q * 4 + m
                xq = xq_tiles[q]
                psv = paux.tile([128, 512], f32, tag="paux", name="psv")
                for k in range(KC):
                    nc.tensor.matmul(psv[:], xq[:, k, m * 128:(m + 1) * 128],
                                     wv_sb[:, k, :], start=(k == 0), stop=(k == KC - 1))
                nc.vector.tensor_copy(out=v_aug[:, st, :, 0:DK],
                                      in_=psv[:].rearrange("p (h d) -> p h d", h=HPC))

            # rope adds are pipelined one chunk behind their swap DMAs
            pending_add = []

            def flush_rope_add():
                while pending_add:
                    dst, a0, a1 = pending_add.pop()
                    nc.vector.tensor_add(dst, a0, a1)

            def qk_task(q, m, swap_sync_ok=True):
                sl = slice(q * 512, (q + 1) * 512)
                xq = xq_tiles[q]
                psqk = paux.tile([128, 512], f32, tag="paux", name="psqk")
                for k in range(KC):
                    nc.tensor.matmul(psqk[:], wqk_sb[:, k, m * 128:(m + 1) * 128],
                                     xq[:, k, :], start=(k == 0), stop=(k == KC - 1))
                t12 = tp2.tile([128, 2, 512], bf16, tag="t12", name="t12")
                s2 = s2p.tile([128, 512], bf16, tag="s2", name="s2")
                nc.vector.tensor_mul(t12[:, 0, :], psqk[:], cos_sb[:, sl])
                nc.vector.tensor_mul(s2[:], psqk[:], sin_sb[:, sl])
                # partner-group swap (evens<->odds), bf16 SBUF-SBUF DMA;
                # alternate queues (when sync isn't load-saturated) so the
                # 32KB copies pipeline across engines
                dma_q = nc.gpsimd if (m % 2 == 0 or not swap_sync_ok) else nc.sync
                for gq in range(4):
                    a, bb = 32 * gq, 32 * (gq ^ 1)
                    dma_q.dma_start(out=t12[a:a + 32, 1, :], in_=s2[bb:bb + 32, :])
                flush_rope_add()
                pending_add.append((qkrot[:, m, sl], t12[:, 0, :], t12[:, 1, :]))

            def oproj_task(n, mo):
                sl = slice(n * 512, (n + 1) * 512)
                pso_t = paux.tile([128, 512], f32, tag="paux", name="pso")
                for kc2 in range(HD // 128):
                    nc.tensor.matmul(pso_t[:], wo_sb[:, kc2, mo * 128:(mo + 1) * 128],
                                     attn_outT[:, kc2, sl],
                                     start=(kc2 == 0), stop=(kc2 == HD // 128 - 1))
                ob = obp.tile([128, 512], f32, tag="ob", name="ob")
                nc.vector.tensor_copy(out=ob[:], in_=pso_t[:])
                nc.sync.dma_start(out=out_d[mo * 128:(mo + 1) * 128, sl], in_=ob[:])

            pending_norm = []

            def normalize_head(j, hp, ps_oA, ps_oB):
                # Copy the unnormalized AV result out of PSUM right away
                # (frees the ps_o bank for the next head pair) and stage both
                # heads' denominators (row DK) into a DRAM bounce buffer.
                unrm = unp.tile([128, 2, 512], bf16, tag="unrm", name="unrm")
                nc.vector.tensor_copy(out=unrm[0:DK, 0, :], in_=ps_oA[0:DK, :])
                nc.vector.tensor_copy(out=unrm[0:DK, 1, :], in_=ps_oB[0:DK, :])
                stage = nrmp.tile([1, 2, 512], f32, tag="stage", name="stage")
                nc.vector.tensor_copy(out=stage[0:1, 0, :], in_=ps_oA[DK:DK + 1, :])
                nc.vector.tensor_copy(out=stage[0:1, 1, :], in_=ps_oB[DK:DK + 1, :])
                d1 = nrmd_pool.tile([1, 1024], f32, tag="d1", name="d1")
                nc.sync.dma_start(out=d1[0:1, :], in_=stage[0:1, :, :])
                pending_norm.append((j, hp, unrm, d1))

            def normalize_tail():
                # Deferred so the DVE reciprocal / muls don't block the DVE
                # FIFO while the DRAM bounce is in flight. The bounce
                # transposes [1, 1024] -> [128, 8] so the reciprocal is
                # partition-parallel; recB is a DMA broadcast back to [64,512].
                while pending_norm:
                    j, hp, unrm, d1 = pending_norm.pop(0)
                    sl = slice(j * 512, (j + 1) * 512)
                    r128 = nrmp.tile([128, 8], f32, tag="r128", name="r128")
                    nc.sync.dma_start(out=r128[:], in_=d1[0, :].rearrange("(p f) -> p f", f=8))
                    r128b = nrmp.tile([128, 8], f32, tag="r128b", name="r128b")
                    with nc.allow_low_precision(reason="softmax reciprocal"):
                        nc.vector.reciprocal(out=r128b[:], in_=r128[:])
                    d2 = nrmd_pool.tile([1, 1024], f32, tag="d2", name="d2")
                    nc.sync.dma_start(out=d2[0, :].rearrange("(p f) -> p f", f=8), in_=r128b[:])
                    for hh, h in ((0, 0), (64, 1)):
                        recB = nrmp.tile([64, 512], f32, tag=f"recB{hh}", name="recB")
                        src = d2[0, hh * 8:hh * 8 + 512]
                        src_b = bass.AP(tensor=src.tensor, offset=src.offset,
                                        ap=[[0, 64]] + list(src.ap))
                        nc.sync.dma_start(out=recB[:], in_=src_b)
                        nc.vector.tensor_mul(attn_outT[hh:hh + 64, hp, sl],
                                             unrm[0:DK, h, :], recB[:])

            # ---------------- prologue ----------------------------------
            # weights stream on the sync queue while x streams on gpsimd so
            # the first V matmuls can start a few us in. QK chunks are
            # ordered [4,0,5,1,...] so the chunks attention hp=0 needs
            # (stationary 4, moving 0) finish their RoPE first.
            QK_ORDER = [4, 0, 5, 1, 6, 2, 7, 3]
            for k in range(KC):
                nc.sync.dma_start(out=wv_sb[:, k, :], in_=wv_d[k * 128:(k + 1) * 128, :])
            load_xq(0)
            for k in range(KC):
                nc.sync.dma_start(out=wqk_sb[:, k, :], in_=wqk_d[k * 128:(k + 1) * 128, :])
            nc.sync.dma_start(out=cos_sb[:], in_=cos_d[:])
            nc.sync.dma_start(out=sin_sb[:], in_=sin_d[:])
            nc.sync.dma_start(out=tri_sb[:], in_=tri_d[:])
            nc.sync.dma_start(out=v_aug[:, :, :, DK],
                              in_=vone_d[:].rearrange("p (a b) -> p a b", b=HPC))
            load_xq(1)
            for m in range(4):
                v_task(0, m)
            for m in QK_ORDER:
                qk_task(0, m, swap_sync_ok=False)
            flush_rope_add()
            # wo is first needed by oproj(0) in quarter 1 - load it after the
            # prologue, in contiguous per-chunk DMAs (a single rearranged DMA
            # lowers to a strided gather that takes ~14us)
            for k in range(HD // 128):
                nc.sync.dma_start(out=wo_sb[:, k, :],
                                  in_=wo_d[k * 128:(k + 1) * 128, :])

            # ---------------- main: attention w/ interleaved fillers ----
            for j in range(NQ):
                if j + 2 <= NQ - 1:
                    load_xq(j + 2)
                fillers = []
                if j + 1 <= NQ - 1:
                    fillers += [(v_task, (j + 1, m)) for m in range(4)]
                    fillers += [(qk_task, (j + 1, m)) for m in QK_ORDER]
                # quarters 0-2 are PE-bound while quarter 3 is exp-bound
                # with PE slack: push all but the first output projection
                # into quarter 3 to rebalance
                if j == 1:
                    fillers += [(oproj_task, (0, mo)) for mo in range(KC)]
                elif j == NQ - 1:
                    fillers += [(oproj_task, (1, mo)) for mo in range(KC)]
                    fillers += [(oproj_task, (2, mo)) for mo in range(KC)]

                n_i = 4 * j + 4
                n_att = 4 * n_i
                # spread fillers across attention iterations; in the last
                # quarter keep a few oproj(j-1) tasks for the post-loop drain
                # so the PE has work while the final softmax normalization
                # chain (DRAM reciprocal bounce) is in flight
                n_spread = len(fillers) - (8 if j == NQ - 1 else 0)
                stride = max(1, n_att // max(1, n_spread))
                fidx = 0
                it = 0

                for hp in range(4):
                    ps_oA = pso.tile([DK + 1, 512], f32, tag="ps_oA", name="ps_oA")
                    ps_oB = pso.tile([DK + 1, 512], f32, tag="ps_oB", name="ps_oB")
                    exs = {}

                    def emit_av(i):
                        ex, off, w = exs.pop(i)
                        nc.tensor.matmul(ps_oA[:, off:512], v_aug[:, i, 2 * hp, :],
                                         ex[:, 0, 0:w],
                                         start=(i == 0), stop=(i == n_i - 1))
                        nc.tensor.matmul(ps_oB[:, off:512], v_aug[:, i, 2 * hp + 1, :],
                                         ex[:, 1, 0:w],
                                         start=(i == 0), stop=(i == n_i - 1))

                    for i in range(n_i):
                        if i == 2:
                            normalize_tail()
                        dd = i - 4 * j
                        off = 128 * dd if dd > 0 else 0
                        w = 512 - off
                        isl = slice(i * 128, (i + 1) * 128)
                        qsl = slice(j * 512 + off, (j + 1) * 512)
                        # both heads packed into disjoint PE row groups ->
                        # the two matmuls run concurrently in the array
                        ps_s = pss.tile([128, 2, 512], f32, tag="ps_s", name="ps_s")
                        nc.tensor.matmul(ps_s[:, 0, 0:w], qkrot[0:64, 4 + hp, isl],
                                         qkrot[0:64, hp, qsl], start=True, stop=True)
                        nc.tensor.matmul(ps_s[:, 1, 0:w], qkrot[64:128, 4 + hp, isl],
                                         qkrot[64:128, hp, qsl], start=True, stop=True)
                        ex = exp_pool.tile([128, 2, 512], bf16, tag="ex", name="ex")
                        nc.scalar.activation(out=ex[:, :, 0:w], in_=ps_s[:, :, 0:w],
                                             func=ActF.Exp, scale=1.0 / math.sqrt(DK))
                        if dd >= 0:
                            # triangular 128-col sub-block of the diagonal tile
                            m = tri_sb[:]
                            m2 = bass.AP(tensor=m.tensor, offset=m.offset,
                                         ap=[m.ap[0], [0, 2], m.ap[1]])
                            nc.vector.tensor_tensor(ex[:, :, 0:128], ex[:, :, 0:128],
                                                    m2, op=Alu.mult)
                        exs[i] = (ex, off, w)
                        if fidx < len(fillers) and it % stride == stride - 1:
                            fn, args = fillers[fidx]
                            fn(*args)
                            fidx += 1
                        it += 1
                        if i >= 2:
                            emit_av(i - 2)
                    if n_i >= 2:
                        emit_av(n_i - 2)
                    emit_av(n_i - 1)
                    normalize_head(j, hp, ps_oA, ps_oB)
                # drain leftover fillers first: their PSUM-freeing DVE copies
                # must precede the normalize reciprocal (which blocks the DVE
                # FIFO on a DRAM round-trip), else the PE stalls on pool WAR
                while fidx < len(fillers):
                    fn, args = fillers[fidx]
                    fn(*args)
                    fidx += 1
                flush_rope_add()
                normalize_tail()

            # ---------------- epilogue: last output projection ----------
            for mo in range(KC):
                oproj_task(NQ - 1, mo)

    nc.compile()
    return nc


def _host_inputs(x, Wq, Wk, Wv, Wo, token_positions):
    bf = ml_dtypes.bfloat16
    x = np.asarray(x, dtype=np.float32)
    Wq = np.asarray(Wq, dtype=np.float32)
    Wk = np.asarray(Wk, dtype=np.float32)
    Wv = np.asarray(Wv, dtype=np.float32)
    Wo = np.asarray(Wo, dtype=np.float32)
    pos = np.asarray(token_positions, dtype=np.float32)

    half = DK // 2
    inv_freq = THETA ** (-(np.arange(half, dtype=np.float32) * 2.0) / DK)  # [32]
    ang = pos[None, :] * inv_freq[:, None]                                  # [32, S]
    cos32 = np.cos(ang).astype(np.float32)
    sin32 = np.sin(ang).astype(np.float32)
    cosT = np.tile(cos32, (4, 1)).astype(bf)                                # [128, S]
    # group-swapped sign pattern: t2[r] = (psqk*sinP)[partner(r)] must equal
    # psqk[partner(r)] * sinT[r] with sinT = [-s, s, -s, s] -> sinP = [s, -s, s, -s]
    sinP = np.concatenate([sin32, -sin32, sin32, -sin32], axis=0).astype(bf)

    # triangular mask for the diagonal 128x128 sub-block: keep key<=query
    p_idx = np.arange(128)[:, None]
    c_idx = np.arange(128)[None, :]
    tri = (p_idx <= c_idx).astype(bf)

    perm = np.concatenate([np.arange(0, DK, 2), np.arange(1, DK, 2)])       # evens|odds
    perm_all = (np.arange(HPC)[:, None] * DK + perm[None, :]).reshape(-1)   # [512]

    in_maps = []
    for c in range(N_CORES):
        b, g = c // 2, c % 2
        rows = slice(g * HD, (g + 1) * HD)
        wqT = np.ascontiguousarray(Wq[rows].T)[:, perm_all]                 # [1024, 512]
        wkT = np.ascontiguousarray(Wk[rows].T)[:, perm_all]
        wqkT = np.ascontiguousarray(np.concatenate([wqT, wkT], axis=1)).astype(bf)
        wvT = np.ascontiguousarray(Wv[rows].T).astype(bf)                   # [1024, 512]
        woT = np.ascontiguousarray(Wo[:, rows].T).astype(bf)                # [512, 1024]
        in_maps.append({
            "xT": np.ascontiguousarray(x[b].T).astype(bf),
            "wqkT": wqkT,
            "wvT": wvT,
            "woT": woT,
            "cosT": cosT,
            "sinT": sinP,
            "tri": tri,
            "vones": np.ones((128, ST * HPC), bf),
        })
    return in_maps


def run(inputs, trace=False):
    """Build (cached), run on 8 cores, return (output, BassKernelResults)."""
    if "nc" not in _prog_cache:
        _prog_cache["nc"] = _build_program()
    nc = _prog_cache["nc"]
    in_maps = _host_inputs(inputs["x"], inputs["Wq"], inputs["Wk"],
                           inputs["Wv"], inputs["Wo"], inputs["token_positions"])
    res = run_bass_kernel_spmd(nc, in_maps, core_ids=list(range(N_CORES)), trace=trace)
    out = np.empty((B, S, D), dtype=np.float32)
    for b in range(B):
        acc = res.results[2 * b]["outT"] + res.results[2 * b + 1]["outT"]
        out[b] = acc.T
    return out, res


def kernel(**inputs) -> np.ndarray:
    out, _ = run(inputs, trace=False)
    return out


# revision 47
# speedup vs baseline: 1.0048x; 1.0048x over previous
"""Causal multi-head attention with RoPE on 8 Trainium2 NeuronCores.

Sharding: core c -> batch b = c//2, head-group g = c%2 (8 of 16 heads).
Each core computes q/k/v projections for its batch+heads, applies RoPE
(evens/odds row-permuted layout so the pair-rotation becomes a
32-partition-group swap done by SBUF-SBUF DMA), runs flash-style causal
attention with transposed scores (softmax sum via an appended ones
column of V -> denominator row in the AV PSUM tile), and a partial
output projection over its head group. Host sums the two per-batch
partials.

v3 (500us -> ~305us): bf16 matmul data (enables fast weight loads,
halves DMA/SBUF), trapezoid causal structure (fully-masked 128-col
sub-blocks of diagonal tiles skipped entirely; only the triangular
sub-block gets a mask multiply), and software-pipelined engine
scheduling: projection matmuls of quarter j+1 and output-projection
matmuls of earlier quarters are interleaved between attention
iterations of quarter j so the tensor engine stays dense (HAM clock
at 2.4GHz) while the scalar engine streams the exp chain; output
projections are biased into the exp-bound last quarter. A warmup
matmul burst opens the HAM clock gate during the initial DMAs; deep
tile-pool buffering keeps WAR semaphores off the critical path.
fp8(e4m3) variants of the attention path were tested and rejected:
quantizing ex or V to fp8 costs 3-5e-2 max-normalized error (the
dominant softmax weights / dominant V entries carry ~2-3% element
error straight into peaked attention rows), over the 2e-2 gate.
"""
import math
import sys

sys.path.insert(0, "/opt/trn_rl_repo")

import numpy as np
import ml_dtypes

import concourse.tile as tile
import concourse.bass as bass
from concourse import bacc, mybir
from concourse.bass_utils import run_bass_kernel_spmd

NUM_HEADS = 16
B, S, D = 4, 2048, 1024
HPC = 8            # heads per core
DK = 64
HD = HPC * DK      # 512 head dims per core
THETA = 10000.0
N_CORES = 8
KC = D // 128      # 8 contraction chunks for projections
NQ = 4             # s-quarters of 512
ST = S // 128      # 16 s-tiles

f32 = mybir.dt.float32
bf16 = mybir.dt.bfloat16
ActF = mybir.ActivationFunctionType
Alu = mybir.AluOpType

_prog_cache = {}


def _build_program():
    nc = bacc.Bacc("TRN2", target_bir_lowering=False, debug=False,
                   enable_asserts=False, num_devices=N_CORES)
    xT_d = nc.dram_tensor("xT", [D, S], bf16, kind="ExternalInput").ap()
    wqk_d = nc.dram_tensor("wqkT", [D, 2 * HD], bf16, kind="ExternalInput").ap()
    wv_d = nc.dram_tensor("wvT", [D, HD], bf16, kind="ExternalInput").ap()
    wo_d = nc.dram_tensor("woT", [HD, D], bf16, kind="ExternalInput").ap()
    cos_d = nc.dram_tensor("cosT", [128, S], bf16, kind="ExternalInput").ap()
    sin_d = nc.dram_tensor("sinT", [128, S], bf16, kind="ExternalInput").ap()
    tri_d = nc.dram_tensor("tri", [128, 128], bf16, kind="ExternalInput").ap()
    vone_d = nc.dram_tensor("vones", [128, ST * HPC], bf16, kind="ExternalInput").ap()
    out_d = nc.dram_tensor("outT", [D, S], f32, kind="ExternalOutput").ap()

    with tile.TileContext(nc) as tc:
        with tc.tile_pool(name="persist", bufs=1) as pp, \
             tc.tile_pool(name="xq", bufs=2) as xp, \
             tc.tile_pool(name="s2", bufs=3) as s2p, \
             tc.tile_pool(name="t12", bufs=3) as tp2, \
             tc.tile_pool(name="ex", bufs=6) as exp_pool, \
             tc.tile_pool(name="unrm", bufs=3) as unp, \
             tc.tile_pool(name="nrm", bufs=3) as nrmp, \
             tc.tile_pool(name="ob", bufs=3) as obp, \
             tc.tile_pool(name="nrmd", bufs=6, space="DRAM") as nrmd_pool, \
             tc.tile_pool(name="paux", bufs=2, space="PSUM") as paux, \
             tc.tile_pool(name="pss", bufs=2, space="PSUM") as pss, \
             tc.tile_pool(name="pso", bufs=1, space="PSUM") as pso:

            # ---------------- persistent SBUF ---------------------------
            qkrot = pp.tile([128, KC, S], bf16, tag="qkrot")           # 32KB/P
            v_aug = pp.tile([128, ST, HPC, DK + 1], bf16, tag="v_aug")  # 16.6KB
            attn_outT = pp.tile([128, NQ, S], bf16, tag="attn_outT")   # 16KB
            wqk_sb = pp.tile([128, KC, 2 * HD], bf16, tag="wqk")       # 16KB
            wv_sb = pp.tile([128, KC, HD], bf16, tag="wv")             # 8KB
            wo_sb = pp.tile([128, HD // 128, D], bf16, tag="wo")       # 8KB
            cos_sb = pp.tile([128, S], bf16, tag="cos")                # 4KB
            sin_sb = pp.tile([128, S], bf16, tag="sin")                # 4KB
            tri_sb = pp.tile([128, 128], bf16, tag="tri")

            # warmup burst: ~6us of dummy matmuls on zeros so the PE HAM
            # clock gate opens (1.2 -> 2.4 GHz) while the first x/weight
            # DMAs are still in flight
            warm_sb = pp.tile([128, 512], bf16, tag="warm")
            nc.gpsimd.memset(warm_sb[:], 0.0)
            for _ in range(14):
                psw = paux.tile([128, 512], f32, tag="paux", name="psw")
                nc.tensor.matmul(psw[:], warm_sb[:, 0:128], warm_sb[:],
                                 start=True, stop=True)

            xq_tiles = {}

            def load_xq(q):
                xq = xp.tile([128, KC, 512], bf16, tag="xq", name=f"xq{q}")
                sl = slice(q * 512, (q + 1) * 512)
                for k in range(KC):
                    nc.gpsimd.dma_start(out=xq[:, k, :], in_=xT_d[k * 128:(k + 1) * 128, sl])
                xq_tiles[q] = xq

            def v_task(q, m):
                st = q * 4 + m
                xq = xq_tiles[q]
                psv = paux.tile([128, 512], f32, tag="paux", name="psv")
                for k in range(KC):
                    nc.tensor.matmul(psv[:], xq[:, k, m * 128:(m + 1) * 128],
                                     wv_sb[:, k, :], start=(k == 0), stop=(k == KC - 1))
                nc.vector.tensor_copy(out=v_aug[:, st, :, 0:DK],
                                      in_=psv[:].rearrange("p (h d) -> p h d", h=HPC))

            # rope adds are pipelined one chunk behind their swap DMAs
            pending_add = []

            def flush_rope_add():
                while pending_add:
                    dst, a0, a1 = pending_add.pop()
                    nc.vector.tensor_add(dst, a0, a1)

            def qk_task(q, m, swap_sync_ok=True):
                sl = slice(q * 512, (q + 1) * 512)
                xq = xq_tiles[q]
                psqk = paux.tile([128, 512], f32, tag="paux", name="psqk")
                for k in range(KC):
                    nc.tensor.matmul(psqk[:], wqk_sb[:, k, m * 128:(m + 1) * 128],
                                     xq[:, k, :], start=(k == 0), stop=(k == KC - 1))
                t12 = tp2.tile([128, 2, 512], bf16, tag="t12", name="t12")
                s2 = s2p.tile([128, 512], bf16, tag="s2", name="s2")
                nc.vector.tensor_mul(t12[:, 0, :], psqk[:], cos_sb[:, sl])
                nc.vector.tensor_mul(s2[:], psqk[:], sin_sb[:, sl])
                # partner-group swap (evens<->odds), bf16 SBUF-SBUF DMA;
                # alternate queues (when sync isn't load-saturated) so the
                # 32KB copies pipeline across engines
                dma_q = nc.gpsimd if (m % 2 == 0 or not swap_sync_ok) else nc.sync
                for gq in range(4):
                    a, bb = 32 * gq, 32 * (gq ^ 1)
                    dma_q.dma_start(out=t12[a:a + 32, 1, :], in_=s2[bb:bb + 32, :])
                flush_rope_add()
                pending_add.append((qkrot[:, m, sl], t12[:, 0, :], t12[:, 1, :]))

            def oproj_task(n, mo):
                sl = slice(n * 512, (n + 1) * 512)
                pso_t = paux.tile([128, 512], f32, tag="paux", name="pso")
                for kc2 in range(HD // 128):
                    nc.tensor.matmul(pso_t[:], wo_sb[:, kc2, mo * 128:(mo + 1) * 128],
                                     attn_outT[:, kc2, sl],
                                     start=(kc2 == 0), stop=(kc2 == HD // 128 - 1))
                ob = obp.tile([128, 512], f32, tag="ob", name="ob")
                nc.vector.tensor_copy(out=ob[:], in_=pso_t[:])
                nc.sync.dma_start(out=out_d[mo * 128:(mo + 1) * 128, sl], in_=ob[:])

            pending_norm = []

            def normalize_head(j, hp, ps_oA, ps_oB):
                # Copy the unnormalized AV result out of PSUM right away
                # (frees the ps_o bank for the next head pair) and stage both
                # heads' denominators (row DK) into a DRAM bounce buffer.
                unrm = unp.tile([128, 2, 512], bf16, tag="unrm", name="unrm")
                nc.vector.tensor_copy(out=unrm[0:DK, 0, :], in_=ps_oA[0:DK, :])
                nc.vector.tensor_copy(out=unrm[0:DK, 1, :], in_=ps_oB[0:DK, :])
                stage = nrmp.tile([1, 2, 512], f32, tag="stage", name="stage")
                nc.vector.tensor_copy(out=stage[0:1, 0, :], in_=ps_oA[DK:DK + 1, :])
                nc.vector.tensor_copy(out=stage[0:1, 1, :], in_=ps_oB[DK:DK + 1, :])
                d1 = nrmd_pool.tile([1, 1024], f32, tag="d1", name="d1")
                nc.sync.dma_start(out=d1[0:1, :], in_=stage[0:1, :, :])
                pending_norm.append((j, hp, unrm, d1))

            def normalize_tail():
                # Deferred so the DVE reciprocal / muls don't block the DVE
                # FIFO while the DRAM bounce is in flight. The bounce
                # transposes [1, 1024] -> [128, 8] so the reciprocal is
                # partition-parallel; recB is a DMA broadcast back to [64,512].
                while pending_norm:
                    j, hp, unrm, d1 = pending_norm.pop(0)
                    sl = slice(j * 512, (j + 1) * 512)
                    r128 = nrmp.tile([128, 8], f32, tag="r128", name="r128")
                    nc.sync.dma_start(out=r128[:], in_=d1[0, :].rearrange("(p f) -> p f", f=8))
                    r128b = nrmp.tile([128, 8], f32, tag="r128b", name="r128b")
                    with nc.allow_low_precision(reason="softmax reciprocal"):
                        nc.vector.reciprocal(out=r128b[:], in_=r128[:])
                    d2 = nrmd_pool.tile([1, 1024], f32, tag="d2", name="d2")
                    nc.sync.dma_start(out=d2[0, :].rearrange("(p f) -> p f", f=8), in_=r128b[:])
                    for hh, h in ((0, 0), (64, 1)):
                        recB = nrmp.tile([64, 512], f32, tag=f"recB{hh}", name="recB")
                        src = d2[0, hh * 8:hh * 8 + 512]
                        src_b = bass.AP(tensor=src.tensor, offset=src.offset,
                                        ap=[[0, 64]] + list(src.ap))
                        nc.sync.dma_start(out=recB[:], in_=src_b)
                        nc.vector.tensor_mul(attn_outT[hh:hh + 64, hp, sl],
                                             unrm[0:DK, h, :], recB[:])

            # ---------------- prologue ----------------------------------
            # weights stream on the sync queue while x streams on gpsimd so
            # the first V matmuls can start a few us in. QK chunks are
            # ordered [4,0,5,1,...] so the chunks attention hp=0 needs
            # (stationary 4, moving 0) finish their RoPE first.
            QK_ORDER = [4, 0, 5, 1, 6, 2, 7, 3]
            for k in range(KC):
                nc.sync.dma_start(out=wv_sb[:, k, :], in_=wv_d[k * 128:(k + 1) * 128, :])
            load_xq(0)
            for k in range(KC):
                nc.sync.dma_start(out=wqk_sb[:, k, :], in_=wqk_d[k * 128:(k + 1) * 128, :])
            nc.sync.dma_start(out=cos_sb[:], in_=cos_d[:])
            nc.sync.dma_start(out=sin_sb[:], in_=sin_d[:])
            nc.sync.dma_start(out=tri_sb[:], in_=tri_d[:])
            nc.sync.dma_start(out=v_aug[:, :, :, DK],
                              in_=vone_d[:].rearrange("p (a b) -> p a b", b=HPC))
            load_xq(1)
            for m in range(4):
                v_task(0, m)
            for m in QK_ORDER:
                qk_task(0, m, swap_sync_ok=False)
            flush_rope_add()
            # wo is first needed by oproj(0) in quarter 1 - load it after the
            # prologue, in contiguous per-chunk DMAs (a single rearranged DMA
            # lowers to a strided gather that takes ~14us)
            for k in range(HD // 128):
                nc.sync.dma_start(out=wo_sb[:, k, :],
                                  in_=wo_d[k * 128:(k + 1) * 128, :])

            # ---------------- main: attention w/ interleaved fillers ----
            for j in range(NQ):
                if j + 2 <= NQ - 1:
                    load_xq(j + 2)
                fillers = []
                if j + 1 <= NQ - 1:
                    fillers += [(v_task, (j + 1, m)) for m in range(4)]
                    fillers += [(qk_task, (j + 1, m)) for m in QK_ORDER]
                # quarters 0-2 are PE-bound while quarter 3 is exp-bound
                # with PE slack: push all but the first output projection
                # into quarter 3 to rebalance
                if j == 1:
                    fillers += [(oproj_task, (0, mo)) for mo in range(KC)]
                elif j == NQ - 1:
                    fillers += [(oproj_task, (1, mo)) for mo in range(KC)]
                    fillers += [(oproj_task, (2, mo)) for mo in range(KC)]

                n_i = 4 * j + 4
                n_att = 4 * n_i
                # spread fillers across attention iterations; in the last
                # quarter keep a few oproj(j-1) tasks for the post-loop drain
                # so the PE has work while the final softmax normalization
                # chain (DRAM reciprocal bounce) is in flight
                n_spread = len(fillers) - (8 if j == NQ - 1 else 0)
                stride = max(1, n_att // max(1, n_spread))
                fidx = 0
                it = 0

                for hp in range(4):
                    ps_oA = pso.tile([DK + 1, 512], f32, tag="ps_oA", name="ps_oA")
                    ps_oB = pso.tile([DK + 1, 512], f32, tag="ps_oB", name="ps_oB")
                    exs = {}

                    def emit_av(i):
                        ex, off, w = exs.pop(i)
                        nc.tensor.matmul(ps_oA[:, off:512], v_aug[:, i, 2 * hp, :],
                                         ex[:, 0, 0:w],
                                         start=(i == 0), stop=(i == n_i - 1))
                        nc.tensor.matmul(ps_oB[:, off:512], v_aug[:, i, 2 * hp + 1, :],
                                         ex[:, 1, 0:w],
                                         start=(i == 0), stop=(i == n_i - 1))

                    for i in range(n_i):
                        if i == 2:
                            normalize_tail()
                        dd = i - 4 * j
                        off = 128 * dd if dd > 0 else 0
                        w = 512 - off
                        isl = slice(i * 128, (i + 1) * 128)
                        qsl = slice(j * 512 + off, (j + 1) * 512)
                        # both heads packed into disjoint PE row groups ->
                        # the two matmuls run concurrently in the array
                        ps_s = pss.tile([128, 2, 512], f32, tag="ps_s", name="ps_s")
                        nc.tensor.matmul(ps_s[:, 0, 0:w], qkrot[0:64, 4 + hp, isl],
                                         qkrot[0:64, hp, qsl], start=True, stop=True)
                        nc.tensor.matmul(ps_s[:, 1, 0:w], qkrot[64:128, 4 + hp, isl],
                                         qkrot[64:128, hp, qsl], start=True, stop=True)
                        ex = exp_pool.tile([128, 2, 512], bf16, tag="ex", name="ex")
                        nc.scalar.activation(out=ex[:, :, 0:w], in_=ps_s[:, :, 0:w],
                                             func=ActF.Exp, scale=1.0 / math.sqrt(DK))
                        if dd >= 0:
                            # triangular 128-col sub-block of the diagonal tile
                            m = tri_sb[:]
                            m2 = bass.AP(tensor=m.tensor, offset=m.offset,
                                         ap=[m.ap[0], [0, 2], m.ap[1]])
                            nc.vector.tensor_tensor(ex[:, :, 0:128], ex[:, :, 0:128],
                                                    m2, op=Alu.mult)
                        exs[i] = (ex, off, w)
                        if fidx < len(fillers) and it % stride == stride - 1:
                            fn, args = fillers[fidx]
                            fn(*args)
                            fidx += 1
                        it += 1
                        if i >= 2:
                            emit_av(i - 2)
                    if n_i >= 2:
                        emit_av(n_i - 2)
                    emit_av(n_i - 1)
                    normalize_head(j, hp, ps_oA, ps_oB)
                # start the last head pair's normalization chain, then drain
                # leftover fillers (their matmuls overlap the DRAM bounce)
                normalize_tail()
                while fidx < len(fillers):
                    fn, args = fillers[fidx]
                    fn(*args)
                    fidx += 1
                flush_rope_add()

            # ---------------- epilogue: last output projection ----------
            for mo in range(KC):
                oproj_task(NQ - 1, mo)

    nc.compile()
    return nc


def _host_inputs(x, Wq, Wk, Wv, Wo, token_positions):
    bf = ml_dtypes.bfloat16
    x = np.asarray(x, dtype=np.float32)
    Wq = np.asarray(Wq, dtype=np.float32)
    Wk = np.asarray(Wk, dtype=np.float32)
    Wv = np.asarray(Wv, dtype=np.float32)
    Wo = np.asarray(Wo, dtype=np.float32)
    pos = np.asarray(token_positions, dtype=np.float32)

    half = DK // 2
    inv_freq = THETA ** (-(np.arange(half, dtype=np.float32) * 2.0) / DK)  # [32]
    ang = pos[None, :] * inv_freq[:, None]                                  # [32, S]
    cos32 = np.cos(ang).astype(np.float32)
    sin32 = np.sin(ang).astype(np.float32)
    cosT = np.tile(cos32, (4, 1)).astype(bf)                                # [128, S]
    # group-swapped sign pattern: t2[r] = (psqk*sinP)[partner(r)] must equal
    # psqk[partner(r)] * sinT[r] with sinT = [-s, s, -s, s] -> sinP = [s, -s, s, -s]
    sinP = np.concatenate([sin32, -sin32, sin32, -sin32], axis=0).astype(bf)

    # triangular mask for the diagonal 128x128 sub-block: keep key<=query
    p_idx = np.arange(128)[:, None]
    c_idx = np.arange(128)[None, :]
    tri = (p_idx <= c_idx).astype(bf)

    perm = np.concatenate([np.arange(0, DK, 2), np.arange(1, DK, 2)])       # evens|odds
    perm_all = (np.arange(HPC)[:, None] * DK + perm[None, :]).reshape(-1)   # [512]

    in_maps = []
    for c in range(N_CORES):
        b, g = c // 2, c % 2
        rows = slice(g * HD, (g + 1) * HD)
        wqT = np.ascontiguousarray(Wq[rows].T)[:, perm_all]                 # [1024, 512]
        wkT = np.ascontiguousarray(Wk[rows].T)[:, perm_all]
        wqkT = np.ascontiguousarray(np.concatenate([wqT, wkT], axis=1)).astype(bf)
        wvT = np.ascontiguousarray(Wv[rows].T).astype(bf)                   # [1024, 512]
        woT = np.ascontiguousarray(Wo[:, rows].T).astype(bf)                # [512, 1024]
        in_maps.append({
            "xT": np.ascontiguousarray(x[b].T).astype(bf),
            "wqkT": wqkT,
            "wvT": wvT,
            "woT": woT,
            "cosT": cosT,
            "sinT": sinP,
            "tri": tri,
            "vones": np.ones((128, ST * HPC), bf),
        })
    return in_maps


def run(inputs, trace=False):
    """Build (cached), run on 8 cores, return (output, BassKernelResults)."""
    if "nc" not in _prog_cache:
        _prog_cache["nc"] = _build_program()
    nc = _prog_cache["nc"]
    in_maps = _host_inputs(inputs["x"], inputs["Wq"], inputs["Wk"],
                           inputs["Wv"], inputs["Wo"], inputs["token_positions"])
    res = run_bass_kernel_spmd(nc, in_maps, core_ids=list(range(N_CORES)), trace=trace)
    out = np.empty((B, S, D), dtype=np.float32)
    for b in range(B):
        acc = res.results[2 * b]["outT"] + res.results[2 * b + 1]["outT"]
        out[b] = acc.T
    return out, res


def kernel(**inputs) -> np.ndarray:
    out, _ = run(inputs, trace=False)
    return out
